# revision 6
# baseline (speedup 1.0000x reference)
"""Distributed multi-head self-attention for Trainium2 (8 NeuronCores).

Problem: b=4, n=2048, dim=1024, heads=16, dim_head=64.
  q = x@Wq; k,v = split(x@Wkv, 2); out = softmax(q k^T / 8) v; y = out@Wout + bout

Sharding: core c <-> (batch b=c//2, head-group g=c%2). Each core computes
q/k/v + attention for its batch's 8 heads (tensor-parallel columns of
Wq/Wkv), then a PARTIAL output projection with its 512 rows of Wout
(row-parallel). Pairs (b,0)/(b,1) combine partials with chunked f32
ReduceScatter (4 chunks of 512 tokens, overlapped with attention(3));
rank g of each pair receives tokens [iq*512+256g, +256) summed. Bias is
pre-added exactly once via sel-masked bias rows opening the PSUM group.

The scalar engine (exp over 33.5M scores/core, ~1.33ns/col) is the
roofline; everything else is scheduled to keep it fed:
 - attention(0) starts after only x chunks 0-3 + Wq/Wk are loaded; the
   rest of x (+Wv/Wout) DMAs, casts, PE transposes and qk-projection
   chunks drip into attention(0)'s step hooks (producers always emitted
   before consumers - engine queues execute in order).
 - attention(0) runs a deep pend queue (Q=9) so its v-matmuls and half
   its v-projections shift out of the PE-choked iq=0 into iq=1.
 - qkproj(p+1) drips 3 matmuls/step into attention(p) instead of
   stalling ACT with a 13us PE blob between pairs.
 - partial out-proj chunks drip into attention(3) behind each iq's
   finalize; only chunk 3 + its ReduceScatter sit on the tail.
TensorEngine math is bf16 with f32 PSUM; scores run two heads
concurrently via tile_position row groups; softmax skips max-subtraction
(scores ~N(0,1)); denominators ride as a 65th ones-row of v; lazy
normalization off the critical path (reciprocal + gpsimd broadcast).
PSUM budget: psS 2x[128,1024]f32 (4 banks) + psO 2x[65,512] (2) +
psP 1x[128,512] (1) + pst 1x[128,256]bf16 transpose ping-pong (1) = 8.
"""

import numpy as np

import concourse.mybir as mybir
import concourse.tile as tile
from concourse import bacc, bass_utils
from concourse.masks import make_identity

N_CORES = 8
B, N, D = 4, 2048, 1024
GH = 8          # heads per core
DH = 64
IN = GH * DH    # 512 inner dims per core
SCALE = DH ** -0.5
PT = 128
KD = D // PT    # 8 dim tiles
MS = N // PT    # 16 seq tiles
MI = IN // PT   # 4 head-pair tiles per core
SCATTER = N // 8  # 256 token rows each core receives per RS chunk
F32 = mybir.dt.float32
BF16 = mybir.dt.bfloat16
RG = [[0, 1], [2, 3], [4, 5], [6, 7]]

_COMPILED = None


def build():
    nc = bacc.Bacc("TRN2", target_bir_lowering=False, debug=False, num_devices=N_CORES)

    x_ext = nc.dram_tensor("x", [N, D], F32, kind="ExternalInput")
    wq_ext = nc.dram_tensor("wq", [D, IN], F32, kind="ExternalInput")
    wk_ext = nc.dram_tensor("wk", [D, IN], F32, kind="ExternalInput")
    wv_ext = nc.dram_tensor("wv", [D, IN], F32, kind="ExternalInput")
    wout_ext = nc.dram_tensor("wout", [IN, D], F32, kind="ExternalInput")
    bout_ext = nc.dram_tensor("bout", [D], F32, kind="ExternalInput")
    sel_ext = nc.dram_tensor("sel", [1, 2], F32, kind="ExternalInput")
    out_ext = nc.dram_tensor("out", [4 * SCATTER, D], F32, kind="ExternalOutput")

    with tile.TileContext(nc) as tc:
        with (
            tc.tile_pool(name="const", bufs=1) as constp,
            tc.tile_pool(name="wpool", bufs=1) as wpool,
            tc.tile_pool(name="qkv", bufs=1) as qkv,
            tc.tile_pool(name="attout", bufs=1) as attoutp,
            tc.tile_pool(name="xT", bufs=1) as xTp,
            tc.tile_pool(name="stage", bufs=3) as stage,
            tc.tile_pool(name="wstage", bufs=3) as wstage,
            tc.tile_pool(name="xbf", bufs=2) as xbfp,
            tc.tile_pool(name="dram", bufs=1, space="DRAM") as dram,
        ):
            ident = constp.tile([PT, PT], BF16)
            make_identity(nc, ident[:])
            bias_row = constp.tile([1, D], F32)
            nc.sync.dma_start(bias_row[:], bout_ext[None, :])
            sel_row = constp.tile([1, 2], F32)
            nc.sync.dma_start(sel_row[:], sel_ext[:])
            # bias rows masked by pair-rank: rank g adds bias only to the
            # token rows it will receive from the ReduceScatter
            bias_g = [constp.tile([1, D], BF16, name=f"bias_g{g}") for g in range(2)]
            for g in range(2):
                nc.vector.tensor_scalar_mul(
                    bias_g[g][:], bias_row[:], sel_row[:, g:g + 1]
                )
            ones_col = constp.tile([1, PT], BF16)
            nc.gpsimd.memset(ones_col[:], 1.0)

            wq_bf = [wpool.tile([PT, IN], BF16, name=f"wq_bf{k}") for k in range(KD)]
            wk_bf = [wpool.tile([PT, IN], BF16, name=f"wk_bf{k}") for k in range(KD)]
            wv_bf = [wpool.tile([PT, IN], BF16, name=f"wv_bf{k}") for k in range(KD)]
            wo_bf = [wpool.tile([PT, D], BF16, name=f"wo_bf{p}") for p in range(MI)]

            xT = [xTp.tile([PT, N], BF16, name=f"xT{k}") for k in range(KD)]
            qT = [qkv.tile([PT, N], BF16, name=f"qT{m}") for m in range(MI)]
            kT = [qkv.tile([PT, N], BF16, name=f"kT{m}") for m in range(MI)]
            vsb = [qkv.tile([PT, GH, 66], BF16, name=f"v{s}") for s in range(MS)]
            attoutT = [attoutp.tile([PT, N], BF16, name=f"attoutT{p}") for p in range(MI)]

            rs_in = [dram.tile([4 * PT, D], F32, name=f"rs_in{i}") for i in range(4)]
            rs_out = [dram.tile([SCATTER, D], F32, name=f"rs_out{i}") for i in range(4)]

            with (
                tc.tile_pool(name="psS", bufs=2, space="PSUM") as psS,
                tc.tile_pool(name="psO", bufs=2, space="PSUM") as psO,
                tc.tile_pool(name="psP", bufs=1, space="PSUM") as psP,
                tc.tile_pool(name="pst", bufs=1, space="PSUM") as pstp,
                tc.tile_pool(name="attn", bufs=10) as attnp,
                tc.tile_pool(name="fin", bufs=2) as finp,
                tc.tile_pool(name="osb", bufs=3) as osbp,
            ):
                # ---------- emission helpers ----------
                def xdma(s):
                    st = stage.tile([PT, D], F32, name="st", tag="st")
                    nc.sync.dma_start(st[:], x_ext[s * PT:(s + 1) * PT, :])
                    return st

                def xprep(s, st):
                    """cast + transpose + copy chunk s into xT[*][:, s-cols]"""
                    xbf = xbfp.tile([PT, D], BF16, name="xbf", tag="xbf")
                    nc.vector.tensor_copy(xbf[:], st[:])
                    for kk in range(0, KD, 2):
                        pt_ = pstp.tile([PT, 2 * PT], BF16, name="pt_", tag="pt")
                        for h in range(2):
                            nc.tensor.transpose(
                                pt_[:, h * PT:(h + 1) * PT],
                                xbf[:, (kk + h) * PT:(kk + h + 1) * PT],
                                ident[:],
                            )
                            nc.vector.tensor_copy(
                                xT[kk + h][:, s * PT:(s + 1) * PT],
                                pt_[:, h * PT:(h + 1) * PT],
                            )

                def wdma_cast(ext, dst, k, cols=IN, gp=False):
                    wst = wstage.tile([PT, cols], F32, name="wst", tag="wst")
                    nc.sync.dma_start(wst[:], ext[k * PT:(k + 1) * PT, :])
                    if gp:
                        nc.gpsimd.tensor_copy(dst[:], wst[:])
                    else:
                        nc.vector.tensor_copy(dst[:], wst[:])

                qkproj_ph = [None]

                def qkproj_mm(p, ch, i):
                    """i-th item (0..17) of pair p's chunk ch: 8 q-mms,
                    q-copy, 8 k-mms, k-copy"""
                    if i == 8 or i == 17:
                        dstT = qT if i == 8 else kT
                        nc.vector.tensor_copy(
                            dstT[p][:, ch * 512:(ch + 1) * 512], qkproj_ph[0][:]
                        )
                        return
                    k = i if i < 8 else i - 9
                    w_bf = wq_bf if i < 8 else wk_bf
                    if k == 0:
                        qkproj_ph[0] = psP.tile([PT, 512], F32, name="ph", tag="psP")
                    nc.tensor.matmul(
                        qkproj_ph[0][:],
                        w_bf[k][:, p * PT:(p + 1) * PT],
                        xT[k][:, ch * 512:(ch + 1) * 512],
                        start=(k == 0), stop=(k == KD - 1),
                    )

                def qkproj_chunk(p, ch):
                    for i in range(18):
                        qkproj_mm(p, ch, i)

                def vproj_s(s):
                    pv = psP.tile([PT, 512], F32, name="pv", tag="psP")
                    for k in range(KD):
                        nc.tensor.matmul(
                            pv[:],
                            xT[k][:, s * PT:(s + 1) * PT],
                            wv_bf[k][:],
                            start=(k == 0), stop=(k == KD - 1),
                        )
                    nc.gpsimd.memset(vsb[s][:, :, 64:65], 1.0)
                    nc.vector.tensor_copy(
                        vsb[s][:, :, 0:64],
                        pv[:].rearrange("p (h e) -> p h e", h=GH),
                    )

                # ---------- partial out-projection + ReduceScatter ----------
                proj_ps = [None]

                def proj_items(iq):
                    """work items for partial y over tokens [iq*512,(iq+1)*512)"""
                    items = []
                    for mt in range(4):
                        m = iq * 4 + mt
                        bias_bf = bias_g[0] if mt < 2 else bias_g[1]
                        for nn in range(2):
                            def grp(m=m, nn=nn, bias_bf=bias_bf):
                                ps = psP.tile([PT, 512], F32, name="po", tag="psP")
                                proj_ps[0] = ps
                                nc.tensor.matmul(
                                    ps[:], ones_col[:],
                                    bias_bf[:, nn * 512:(nn + 1) * 512],
                                    start=True, stop=False,
                                )
                                for p in range(MI):
                                    nc.tensor.matmul(
                                        ps[:],
                                        attoutT[p][:, m * PT:(m + 1) * PT],
                                        wo_bf[p][:, nn * 512:(nn + 1) * 512],
                                        start=False, stop=(p == MI - 1),
                                    )

                            def cpy(iq=iq, mt=mt, nn=nn, last=(iq == 3)):
                                osb = osbp.tile([PT, 512], F32, name="osb", tag="osb")
                                if last:
                                    nc.scalar.copy(osb[:], proj_ps[0][:])
                                else:
                                    nc.vector.tensor_copy(osb[:], proj_ps[0][:])
                                nc.sync.dma_start(
                                    rs_in[iq][mt * PT:(mt + 1) * PT,
                                              nn * 512:(nn + 1) * 512],
                                    osb[:],
                                )

                            items.append(grp)
                            items.append(cpy)

                    def rs(iq=iq):
                        nc.gpsimd.collective_compute(
                            "ReduceScatter", mybir.AluOpType.add,
                            replica_groups=RG,
                            ins=[rs_in[iq].opt()], outs=[rs_out[iq].opt()],
                        )
                        nc.sync.dma_start(
                            out_ext[iq * SCATTER:(iq + 1) * SCATTER, :],
                            rs_out[iq][:],
                        )

                    items.append(rs)
                    return items

                # ---------- attention ----------
                def attention(p, hook=None, on_finalize=None, Q=1):
                    steps = [(iq, j) for iq in range(4) for j in range(MS)]
                    outs = {}
                    pendq = []
                    for iq, j in steps:
                        ps = psS.tile([PT, 1024], F32, name="ps", tag="psS")
                        nc.tensor.matmul(
                            ps[:, 0:512],
                            kT[p][0:64, j * PT:(j + 1) * PT],
                            qT[p][0:64, iq * 512:(iq + 1) * 512],
                            start=True, stop=True,
                            tile_position=(0, 0),
                        )
                        nc.tensor.matmul(
                            ps[:, 512:1024],
                            kT[p][64:128, j * PT:(j + 1) * PT],
                            qT[p][64:128, iq * 512:(iq + 1) * 512],
                            start=True, stop=True,
                            tile_position=(64, 0),
                        )
                        at = attnp.tile([PT, 1024], BF16, name="at", tag="at")
                        nc.scalar.activation(
                            at[:], ps[:], mybir.ActivationFunctionType.Exp,
                            scale=SCALE,
                        )
                        if hook is not None:
                            hook(iq, j)
                        pendq.append((iq, j, at))
                        if len(pendq) > Q:
                            pop = pendq.pop(0)
                            emit_vmm(p, outs, *pop)
                            if pop[1] == MS - 1:
                                finalize(p, outs, pop[0])
                                if on_finalize is not None:
                                    on_finalize(pop[0])
                    for pop in pendq:
                        emit_vmm(p, outs, *pop)
                        if pop[1] == MS - 1:
                            finalize(p, outs, pop[0])
                            if on_finalize is not None:
                                on_finalize(pop[0])

                def emit_vmm(p, outs, iq, j, at):
                    if j == 0:
                        outs[iq] = (
                            psO.tile([65, 512], F32, name="oA", tag="psO"),
                            psO.tile([65, 512], F32, name="oB", tag="psO"),
                        )
                    oA, oB = outs[iq]
                    nc.tensor.matmul(
                        oA[:], vsb[j][:, 2 * p, 0:65], at[:, 0:512],
                        start=(j == 0), stop=(j == MS - 1),
                    )
                    nc.tensor.matmul(
                        oB[:], vsb[j][:, 2 * p + 1, 0:65], at[:, 512:1024],
                        start=(j == 0), stop=(j == MS - 1),
                    )

                def finalize(p, outs, iq):
                    dens = []
                    for hh, o in enumerate(outs[iq]):
                        seg = attoutT[p][hh * 64:(hh + 1) * 64,
                                         iq * 512:(iq + 1) * 512]
                        nc.vector.tensor_copy(seg, o[0:64, :])
                        den = finp.tile([1, 512], F32, name="den", tag="den")
                        nc.vector.tensor_copy(den[:], o[64:65, :])
                        dens.append((hh, den))
                    for hh, den in dens:
                        recip = finp.tile([1, 512], F32, name="recip", tag="recip")
                        nc.vector.reciprocal_approx_fast(recip[:], den[:])
                        bc = finp.tile([PT, 512], F32, name="bc", tag="bc")
                        nc.gpsimd.partition_broadcast(bc[:], recip[:])
                        seg = attoutT[p][hh * 64:(hh + 1) * 64,
                                         iq * 512:(iq + 1) * 512]
                        nc.vector.tensor_tensor(
                            seg, seg, bc[hh * 64:(hh + 1) * 64, :],
                            op=mybir.AluOpType.mult,
                        )

                # ---------- drip machinery ----------
                drip = []

                def drip_pump(k):
                    for _ in range(min(k, len(drip))):
                        drip.pop(0)()

                def queue_qkproj(p):
                    for ch in range(4):
                        for i in range(18):
                            drip.append(lambda p=p, ch=ch, i=i: qkproj_mm(p, ch, i))

                # ---------- phase 0 head: minimum to start attention(0) ----------
                x_stages = {}
                for s in range(4):
                    x_stages[s] = xdma(s)
                for s in range(4):
                    xprep(s, x_stages.pop(s))
                for k in range(KD):
                    wdma_cast(wq_ext, wq_bf[k], k)
                for k in range(KD):
                    wdma_cast(wk_ext, wk_bf[k], k)
                for k in range(KD):
                    wdma_cast(wv_ext, wv_bf[k], k)
                qkproj_chunk(0, 0)
                for s in range(4, 8):
                    x_stages[s] = xdma(s)

                # ---------- attention(0): drip the rest of phase 0 ----------
                def hook0(iq, j):
                    if iq == 0:
                        c = j + 4
                        if c <= 15:
                            xprep(c, x_stages.pop(c))
                            if c + 4 <= 15:
                                x_stages[c + 4] = xdma(c + 4)
                        if j in (3, 7, 11):
                            qkproj_chunk(0, (j + 1) // 4)
                        vproj_s(j)
                    elif iq == 1:
                        if j == 0:
                            for p_ in range(MI):
                                wdma_cast(wout_ext, wo_bf[p_], p_, cols=D, gp=True)
                    if iq >= 2:
                        drip_pump(3)

                queue_qkproj(1)
                attention(0, hook=hook0, Q=9)
                drip_pump(len(drip))

                queue_qkproj(2)
                attention(1, hook=lambda iq, j: drip_pump(3) if iq >= 1 else None)
                drip_pump(len(drip))

                queue_qkproj(3)
                attention(2, hook=lambda iq, j: drip_pump(3) if iq >= 1 else None)
                drip_pump(len(drip))

                def on_fin3(iq):
                    drip.extend(proj_items(iq))

                attention(3, hook=lambda iq, j: drip_pump(2), on_finalize=on_fin3)
                drip_pump(len(drip))

    nc.compile()
    return nc


def _shard_inputs(x, Wq, Wkv, Wout, bout):
    in_maps = []
    for c in range(N_CORES):
        b, g = c // 2, c % 2
        sel = np.zeros((1, 2), dtype=np.float32)
        sel[0, g] = 1.0
        in_maps.append({
            "x": np.ascontiguousarray(x[b], dtype=np.float32),
            "wq": np.ascontiguousarray(Wq[:, g * IN:(g + 1) * IN], dtype=np.float32),
            "wk": np.ascontiguousarray(Wkv[:, g * IN:(g + 1) * IN], dtype=np.float32),
            "wv": np.ascontiguousarray(
                Wkv[:, D + g * IN:D + (g + 1) * IN], dtype=np.float32
            ),
            "wout": np.ascontiguousarray(Wout[g * IN:(g + 1) * IN, :], dtype=np.float32),
            "bout": np.ascontiguousarray(bout, dtype=np.float32),
            "sel": sel,
        })
    return in_maps


def kernel(x, Wq, Wkv, Wout, bout):
    global _COMPILED
    if _COMPILED is None:
        _COMPILED = build()
    nc = _COMPILED
    in_maps = _shard_inputs(
        np.asarray(x), np.asarray(Wq), np.asarray(Wkv), np.asarray(Wout),
        np.asarray(bout),
    )
    res = bass_utils.run_bass_kernel_spmd(nc, in_maps, core_ids=list(range(N_CORES)))
    out = np.empty((B, N, D), dtype=np.float32)
    for c in range(N_CORES):
        b, g = c // 2, c % 2
        r = res.results[c]["out"]
        for iq in range(4):
            out[b, iq * 512 + g * SCATTER: iq * 512 + (g + 1) * SCATTER, :] = (
                r[iq * SCATTER:(iq + 1) * SCATTER]
            )
    return out


if __name__ == "__main__":
    rng = np.random.default_rng(0)
    x = rng.standard_normal((B, N, D)).astype(np.float32)
    Wq = rng.standard_normal((D, D)).astype(np.float32) * D ** -0.5
    Wkv = rng.standard_normal((D, 2 * D)).astype(np.float32) * D ** -0.5
    Wout = rng.standard_normal((D, D)).astype(np.float32) * D ** -0.5
    bout = np.zeros((D,), dtype=np.float32)
    y = kernel(x=x, Wq=Wq, Wkv=Wkv, Wout=Wout, bout=bout)
    print("out shape:", y.shape, "finite:", np.isfinite(y).all())


# revision 10
# speedup vs baseline: 1.2320x; 1.2320x over previous
"""Distributed multi-head self-attention for Trainium2 (8 NeuronCores).

Problem: b=4, n=2048, dim=1024, heads=16, dim_head=64.
  q = x@Wq; k,v = split(x@Wkv, 2); out = softmax(q k^T / 8) v; y = out@Wout + bout

Sharding: core c <-> (batch b=c//2, head-group g=c%2). Each core computes
q/k/v + attention for its batch's 8 heads (tensor-parallel columns of
Wq/Wkv), then a PARTIAL output projection with its 512 rows of Wout
(row-parallel). Pairs (b,0)/(b,1) exchange partials with a chunked bf16
AllToAll (4 chunks of 512 tokens, overlapped with attention(3)) and sum
the two halves on gpsimd; rank g of each pair owns tokens
[iq*512+256g, +256). Bias is pre-added exactly once via sel-masked bias
rows opening the PSUM group on the owning rank.

The scalar engine (exp over 33.5M scores/core, ~1.33ns/col) is the
roofline; everything else is scheduled to keep it fed:
 - attention(0) starts after only x chunks 0-3 + Wq/Wk are loaded; the
   rest of x (+Wv/Wout) DMAs, f32 PE transposes (cast fused into one
   wide PSUM->SBUF copy per transpose pair) and qk-projection chunks
   drip into attention(0)'s iq=0 step hook (producers always emitted
   before consumers - engine queues execute in order).
 - qkproj(p+1) drips a few matmuls/step into attention(p) instead of
   stalling ACT with a 13us PE blob between pairs.
 - partial out-proj chunks + AllToAll + adds drip into attention(3)
   behind each iq's finalize; only chunk 3 sits on the tail.
TensorEngine math is bf16 with f32 PSUM; scores run two heads
concurrently via tile_position row groups; softmax skips max-subtraction
(scores ~N(0,1)); denominators ride as a 65th ones-row of v; lazy
normalization off the critical path (reciprocal + gpsimd broadcast).
PSUM budget: psS 2x[128,1024]f32 (4 banks) + psO 2x[65,512] (2) +
psP 1x[128,512] (1) + one static [128,512]f32 transpose scratch (1) = 8.
"""

import numpy as np

import concourse.mybir as mybir
import concourse.tile as tile
from concourse import bacc, bass_utils
from concourse.masks import make_identity

N_CORES = 8
B, N, D = 4, 2048, 1024
GH = 8          # heads per core
DH = 64
IN = GH * DH    # 512 inner dims per core
SCALE = DH ** -0.5
PT = 128
KD = D // PT    # 8 dim tiles
MS = N // PT    # 16 seq tiles
MI = IN // PT   # 4 head-pair tiles per core
SCATTER = N // 8  # 256 token rows each core owns per chunk
F32 = mybir.dt.float32
BF16 = mybir.dt.bfloat16
RG = [[0, 1], [2, 3], [4, 5], [6, 7]]

_COMPILED = None


def build():
    nc = bacc.Bacc("TRN2", target_bir_lowering=False, debug=False, num_devices=N_CORES)

    x_ext = nc.dram_tensor("x", [N, D], F32, kind="ExternalInput")
    wq_ext = nc.dram_tensor("wq", [D, IN], F32, kind="ExternalInput")
    wk_ext = nc.dram_tensor("wk", [D, IN], F32, kind="ExternalInput")
    wv_ext = nc.dram_tensor("wv", [D, IN], F32, kind="ExternalInput")
    wout_ext = nc.dram_tensor("wout", [IN, D], F32, kind="ExternalInput")
    bout_ext = nc.dram_tensor("bout", [D], F32, kind="ExternalInput")
    sel_ext = nc.dram_tensor("sel", [1, 2], F32, kind="ExternalInput")
    out_ext = nc.dram_tensor("out", [4 * SCATTER, D], F32, kind="ExternalOutput")

    with tile.TileContext(nc) as tc:
        with (
            tc.tile_pool(name="const", bufs=1) as constp,
            tc.tile_pool(name="wpool", bufs=1) as wpool,
            tc.tile_pool(name="qkv", bufs=1) as qkv,
            tc.tile_pool(name="attout", bufs=1) as attoutp,
            tc.tile_pool(name="xT", bufs=1) as xTp,
            tc.tile_pool(name="stage", bufs=4) as stage,
            tc.tile_pool(name="wstage", bufs=3) as wstage,
            tc.tile_pool(name="dram", bufs=1, space="DRAM") as dram,
        ):
            identf = constp.tile([PT, PT], F32)
            make_identity(nc, identf[:])
            bias_row = constp.tile([1, D], F32)
            nc.sync.dma_start(bias_row[:], bout_ext[None, :])
            sel_row = constp.tile([1, 2], F32)
            nc.sync.dma_start(sel_row[:], sel_ext[:])
            # bias rows masked by pair-rank: rank g adds bias only to the
            # token rows it will own after the exchange
            bias_g = [constp.tile([1, D], BF16, name=f"bias_g{g}") for g in range(2)]
            for g in range(2):
                nc.vector.tensor_scalar_mul(
                    bias_g[g][:], bias_row[:], sel_row[:, g:g + 1]
                )
            ones_col = constp.tile([1, PT], BF16)
            nc.gpsimd.memset(ones_col[:], 1.0)

            wq_bf = [wpool.tile([PT, IN], BF16, name=f"wq_bf{k}") for k in range(KD)]
            wk_bf = [wpool.tile([PT, IN], BF16, name=f"wk_bf{k}") for k in range(KD)]
            wv_bf = [wpool.tile([PT, IN], BF16, name=f"wv_bf{k}") for k in range(KD)]
            wo_bf = [wpool.tile([PT, D], BF16, name=f"wo_bf{p}") for p in range(MI)]

            xTall = xTp.tile([PT, KD, N], BF16, name="xTall")
            qT = [qkv.tile([PT, N], BF16, name=f"qT{m}") for m in range(MI)]
            kT = [qkv.tile([PT, N], BF16, name=f"kT{m}") for m in range(MI)]
            vsb = [qkv.tile([PT, GH, 66], BF16, name=f"v{s}") for s in range(MS)]
            attoutT = [attoutp.tile([PT, N], BF16, name=f"attoutT{p}") for p in range(MI)]

            a2a_in = [dram.tile([4 * PT, D], BF16, name=f"a2a_in{i}") for i in range(4)]
            a2a_out = [dram.tile([2 * PT, D], BF16, name=f"a2a_out{i}") for i in range(4)]

            with (
                tc.tile_pool(name="psS", bufs=2, space="PSUM") as psS,
                tc.tile_pool(name="psO", bufs=2, space="PSUM") as psO,
                tc.tile_pool(name="psP", bufs=1, space="PSUM") as psP,
                tc.tile_pool(name="pst", bufs=1, space="PSUM") as pstp,
                tc.tile_pool(name="attn", bufs=6) as attnp,
                tc.tile_pool(name="fin", bufs=2) as finp,
                tc.tile_pool(name="osb", bufs=3) as osbp,
                tc.tile_pool(name="asb", bufs=2) as asbp,
            ):
                pstT = pstp.tile([PT, 512], F32, name="pstT")

                # ---------- emission helpers ----------
                def xdma(s):
                    st = stage.tile([PT, D], F32, name="st", tag="st")
                    nc.sync.dma_start(st[:], x_ext[s * PT:(s + 1) * PT, :])
                    return st

                def xprep(s, st):
                    """f32 PE transpose of chunk s; cast fused into one
                    [128,2,128] PSUM->SBUF copy per transpose pair"""
                    for q in range(4):
                        base = (q % 2) * 256
                        for h in range(2):
                            nc.tensor.transpose(
                                pstT[:, base + h * PT: base + (h + 1) * PT],
                                st[:, (2 * q + h) * PT:(2 * q + h + 1) * PT],
                                identf[:],
                            )
                        nc.vector.tensor_copy(
                            xTall[:, 2 * q:2 * q + 2, s * PT:(s + 1) * PT],
                            pstT[:, base:base + 256].rearrange(
                                "p (a b) -> p a b", a=2
                            ),
                        )

                def wdma_cast(ext, dst, k, cols=IN, gp=False):
                    wst = wstage.tile([PT, cols], F32, name="wst", tag="wst")
                    nc.sync.dma_start(wst[:], ext[k * PT:(k + 1) * PT, :])
                    if gp:
                        nc.gpsimd.tensor_copy(dst[:], wst[:])
                    else:
                        nc.vector.tensor_copy(dst[:], wst[:])

                qkproj_ph = [None]

                def qkproj_mm(p, ch, i):
                    """i-th item (0..17) of pair p's chunk ch: 8 q-mms,
                    q-copy, 8 k-mms, k-copy"""
                    if i == 8 or i == 17:
                        dstT = qT if i == 8 else kT
                        nc.vector.tensor_copy(
                            dstT[p][:, ch * 512:(ch + 1) * 512], qkproj_ph[0][:]
                        )
                        return
                    k = i if i < 8 else i - 9
                    w_bf = wq_bf if i < 8 else wk_bf
                    if k == 0:
                        qkproj_ph[0] = psP.tile([PT, 512], F32, name="ph", tag="psP")
                    nc.tensor.matmul(
                        qkproj_ph[0][:],
                        w_bf[k][:, p * PT:(p + 1) * PT],
                        xTall[:, k, ch * 512:(ch + 1) * 512],
                        start=(k == 0), stop=(k == KD - 1),
                    )

                def qkproj_chunk(p, ch):
                    for i in range(18):
                        qkproj_mm(p, ch, i)

                def vproj_s(s):
                    pv = psP.tile([PT, 512], F32, name="pv", tag="psP")
                    for k in range(KD):
                        nc.tensor.matmul(
                            pv[:],
                            xTall[:, k, s * PT:(s + 1) * PT],
                            wv_bf[k][:],
                            start=(k == 0), stop=(k == KD - 1),
                        )
                    nc.gpsimd.memset(vsb[s][:, :, 64:65], 1.0)
                    nc.vector.tensor_copy(
                        vsb[s][:, :, 0:64],
                        pv[:].rearrange("p (h e) -> p h e", h=GH),
                    )

                # ---------- partial out-proj + AllToAll + pair-sum ----------
                proj_ps = [None]

                def proj_items(iq):
                    """work items for partial y over tokens [iq*512,(iq+1)*512)"""
                    items = []
                    for mt in range(4):
                        m = iq * 4 + mt
                        bias_bf = bias_g[0] if mt < 2 else bias_g[1]
                        for nn in range(2):
                            def grp(m=m, nn=nn, bias_bf=bias_bf):
                                ps = psP.tile([PT, 512], F32, name="po", tag="psP")
                                proj_ps[0] = ps
                                nc.tensor.matmul(
                                    ps[:], ones_col[:],
                                    bias_bf[:, nn * 512:(nn + 1) * 512],
                                    start=True, stop=False,
                                )
                                for p in range(MI):
                                    nc.tensor.matmul(
                                        ps[:],
                                        attoutT[p][:, m * PT:(m + 1) * PT],
                                        wo_bf[p][:, nn * 512:(nn + 1) * 512],
                                        start=False, stop=(p == MI - 1),
                                    )

                            def cpy(iq=iq, mt=mt, nn=nn):
                                osb = osbp.tile([PT, 512], BF16, name="osb", tag="osb")
                                nc.vector.tensor_copy(osb[:], proj_ps[0][:])
                                nc.sync.dma_start(
                                    a2a_in[iq][mt * PT:(mt + 1) * PT,
                                               nn * 512:(nn + 1) * 512],
                                    osb[:],
                                )

                            items.append(grp)
                            items.append(cpy)

                    def rs(iq=iq):
                        nc.gpsimd.collective_compute(
                            "ReduceScatter", mybir.AluOpType.add,
                            replica_groups=RG,
                            ins=[a2a_in[iq].opt()], outs=[a2a_out[iq].opt()],
                        )

                    items.append(rs)

                    # upcast the received summed rows, write out
                    for r in range(2):
                        def post(iq=iq, r=r):
                            h0 = asbp.tile([PT, D], BF16, name="h0", tag="h0")
                            nc.sync.dma_start(
                                h0[:], a2a_out[iq][r * PT:(r + 1) * PT, :]
                            )
                            fs = asbp.tile([PT, D], F32, name="fs", tag="fs")
                            nc.gpsimd.tensor_copy(fs[:], h0[:])
                            nc.sync.dma_start(
                                out_ext[iq * SCATTER + r * PT:
                                        iq * SCATTER + (r + 1) * PT, :],
                                fs[:],
                            )

                        items.append(post)
                    return items

                # ---------- attention ----------
                def attention(p, hook=None, on_finalize=None):
                    steps = [(iq, j) for iq in range(4) for j in range(MS)]
                    outs = {}
                    pend = None
                    for iq, j in steps:
                        ps = psS.tile([PT, 1024], F32, name="ps", tag="psS")
                        nc.tensor.matmul(
                            ps[:, 0:512],
                            kT[p][0:64, j * PT:(j + 1) * PT],
                            qT[p][0:64, iq * 512:(iq + 1) * 512],
                            start=True, stop=True,
                            tile_position=(0, 0),
                        )
                        nc.tensor.matmul(
                            ps[:, 512:1024],
                            kT[p][64:128, j * PT:(j + 1) * PT],
                            qT[p][64:128, iq * 512:(iq + 1) * 512],
                            start=True, stop=True,
                            tile_position=(64, 0),
                        )
                        at = attnp.tile([PT, 1024], BF16, name="at", tag="at")
                        nc.scalar.activation(
                            at[:], ps[:], mybir.ActivationFunctionType.Exp,
                            scale=SCALE,
                        )
                        if hook is not None:
                            hook(iq, j)
                        if pend is not None:
                            emit_vmm(p, outs, *pend)
                            if pend[1] == MS - 1:
                                finalize(p, outs, pend[0])
                                if on_finalize is not None:
                                    on_finalize(pend[0])
                        pend = (iq, j, at)
                    emit_vmm(p, outs, *pend)
                    finalize(p, outs, pend[0])
                    if on_finalize is not None:
                        on_finalize(pend[0])

                def emit_vmm(p, outs, iq, j, at):
                    if j == 0:
                        outs[iq] = (
                            psO.tile([65, 512], F32, name="oA", tag="psO"),
                            psO.tile([65, 512], F32, name="oB", tag="psO"),
                        )
                    oA, oB = outs[iq]
                    nc.tensor.matmul(
                        oA[:], vsb[j][:, 2 * p, 0:65], at[:, 0:512],
                        start=(j == 0), stop=(j == MS - 1),
                    )
                    nc.tensor.matmul(
                        oB[:], vsb[j][:, 2 * p + 1, 0:65], at[:, 512:1024],
                        start=(j == 0), stop=(j == MS - 1),
                    )

                def finalize(p, outs, iq):
                    dens = []
                    for hh, o in enumerate(outs[iq]):
                        seg = attoutT[p][hh * 64:(hh + 1) * 64,
                                         iq * 512:(iq + 1) * 512]
                        nc.vector.tensor_copy(seg, o[0:64, :])
                        den = finp.tile([1, 512], F32, name="den", tag="den")
                        nc.vector.tensor_copy(den[:], o[64:65, :])
                        dens.append((hh, den))
                    for hh, den in dens:
                        recip = finp.tile([1, 512], F32, name="recip", tag="recip")
                        nc.vector.reciprocal_approx_fast(recip[:], den[:])
                        bc = finp.tile([PT, 512], F32, name="bc", tag="bc")
                        nc.gpsimd.partition_broadcast(bc[:], recip[:])
                        seg = attoutT[p][hh * 64:(hh + 1) * 64,
                                         iq * 512:(iq + 1) * 512]
                        nc.vector.tensor_tensor(
                            seg, seg, bc[hh * 64:(hh + 1) * 64, :],
                            op=mybir.AluOpType.mult,
                        )

                # ---------- drip machinery ----------
                drip = []

                def drip_pump(k):
                    for _ in range(min(k, len(drip))):
                        drip.pop(0)()

                def queue_qkproj(p):
                    for ch in range(4):
                        for i in range(18):
                            drip.append(lambda p=p, ch=ch, i=i: qkproj_mm(p, ch, i))

                # ---------- phase 0 head: minimum to start attention(0) ----------
                x_stages = {}
                for s in range(4):
                    x_stages[s] = xdma(s)
                for s in range(4):
                    xprep(s, x_stages.pop(s))
                for k in range(KD):
                    wdma_cast(wq_ext, wq_bf[k], k)
                for k in range(KD):
                    wdma_cast(wk_ext, wk_bf[k], k)
                for k in range(KD):
                    wdma_cast(wv_ext, wv_bf[k], k)
                qkproj_chunk(0, 0)
                for s in range(4, 8):
                    x_stages[s] = xdma(s)

                # ---------- attention(0): drip the rest of phase 0 ----------
                def hook0(iq, j):
                    if iq == 0:
                        c = j + 4
                        if c <= 15:
                            xprep(c, x_stages.pop(c))
                            if c + 4 <= 15:
                                x_stages[c + 4] = xdma(c + 4)
                        if j in (3, 7, 11):
                            qkproj_chunk(0, (j + 1) // 4)
                        vproj_s(j)
                    elif iq == 1 and j == 0:
                        for p_ in range(MI):
                            wdma_cast(wout_ext, wo_bf[p_], p_, cols=D, gp=True)
                    if iq >= 1:
                        drip_pump(3)

                queue_qkproj(1)
                attention(0, hook=hook0)
                drip_pump(len(drip))

                queue_qkproj(2)
                attention(1, hook=lambda iq, j: drip_pump(2))
                drip_pump(len(drip))

                queue_qkproj(3)
                attention(2, hook=lambda iq, j: drip_pump(2))
                drip_pump(len(drip))

                def on_fin3(iq):
                    drip.extend(proj_items(iq))

                attention(3, hook=lambda iq, j: drip_pump(2), on_finalize=on_fin3)
                drip_pump(len(drip))

    nc.compile()
    return nc


def _shard_inputs(x, Wq, Wkv, Wout, bout):
    in_maps = []
    for c in range(N_CORES):
        b, g = c // 2, c % 2
        sel = np.zeros((1, 2), dtype=np.float32)
        sel[0, g] = 1.0
        in_maps.append({
            "x": np.ascontiguousarray(x[b], dtype=np.float32),
            "wq": np.ascontiguousarray(Wq[:, g * IN:(g + 1) * IN], dtype=np.float32),
            "wk": np.ascontiguousarray(Wkv[:, g * IN:(g + 1) * IN], dtype=np.float32),
            "wv": np.ascontiguousarray(
                Wkv[:, D + g * IN:D + (g + 1) * IN], dtype=np.float32
            ),
            "wout": np.ascontiguousarray(Wout[g * IN:(g + 1) * IN, :], dtype=np.float32),
            "bout": np.ascontiguousarray(bout, dtype=np.float32),
            "sel": sel,
        })
    return in_maps


def kernel(x, Wq, Wkv, Wout, bout):
    global _COMPILED
    if _COMPILED is None:
        _COMPILED = build()
    nc = _COMPILED
    in_maps = _shard_inputs(
        np.asarray(x), np.asarray(Wq), np.asarray(Wkv), np.asarray(Wout),
        np.asarray(bout),
    )
    res = bass_utils.run_bass_kernel_spmd(nc, in_maps, core_ids=list(range(N_CORES)))
    out = np.empty((B, N, D), dtype=np.float32)
    for c in range(N_CORES):
        b, g = c // 2, c % 2
        r = res.results[c]["out"]
        for iq in range(4):
            out[b, iq * 512 + g * SCATTER: iq * 512 + (g + 1) * SCATTER, :] = (
                r[iq * SCATTER:(iq + 1) * SCATTER]
            )
    return out


if __name__ == "__main__":
    rng = np.random.default_rng(0)
    x = rng.standard_normal((B, N, D)).astype(np.float32)
    Wq = rng.standard_normal((D, D)).astype(np.float32) * D ** -0.5
    Wkv = rng.standard_normal((D, 2 * D)).astype(np.float32) * D ** -0.5
    Wout = rng.standard_normal((D, D)).astype(np.float32) * D ** -0.5
    bout = np.zeros((D,), dtype=np.float32)
    y = kernel(x=x, Wq=Wq, Wkv=Wkv, Wout=Wout, bout=bout)
    print("out shape:", y.shape, "finite:", np.isfinite(y).all())
